# revision 3
# baseline (speedup 1.0000x reference)
"""EpisodicMemory Trainium2 kernel.

Data-parallel over batch across 8 NeuronCores (128 batch rows per core).
Per-core program (SPMD, weights baked into the NEFF as inline constants):

  Phase 1 (scoring): G[b,s] = sigmoid(fc2(tanh(fc1(feat)))) where
    feat = [C*Q, C*prev_M, |C-Q|, |C-prev_M|].  Computed in a
    feature-on-partitions layout so fc1 is an fp32r matmul with N=512.

  Phase 2 (gated GRU scan over s): per step the PE computes
    [C_s; h] @ [W_ih; W_hh].T with r/z fused (n kept split because r
    multiplies only the hidden half), then DVE/ACT apply
    h_new = h + g*(1-z)*(n - h)  with g a per-partition scalar.

All matmul operands are float32r (TF32-like single-pass PE mode, full
rate at moving dim >= 256); everything else stays fp32.
"""
import numpy as np

H = 512
SH = 120
B = 1024
S = 64
NCORES = 8
BPC = B // NCORES  # 128 batch per core
KH = H // 128      # 4 h-tiles
G3 = 3 * H         # 1536
SGRP = 4           # sentences per scoring group
NGRP = S // SGRP   # 16 groups

_CACHE = {}


def _build(Wt, F1t, F2t, fc1_b, fc2_b, b_ih, b_hh):
    import concourse.bass as bass
    import concourse.tile as tile
    from concourse import bacc, mybir

    FP32 = mybir.dt.float32
    FP32R = mybir.dt.float32r
    AF = mybir.ActivationFunctionType
    OP = mybir.AluOpType

    use_gru_bias = bool(np.any(b_ih != 0) or np.any(b_hh != 0))
    use_fc1_bias = bool(np.any(fc1_b != 0))
    fc2_bias = float(fc2_b[0])

    nc = bacc.Bacc("TRN2", target_bir_lowering=False, debug=False,
                   num_devices=NCORES)

    c_t = nc.dram_tensor("c_t", [S, KH, 128, BPC], FP32R, kind="ExternalInput")
    q_t = nc.dram_tensor("q_t", [KH, 128, BPC], FP32, kind="ExternalInput")
    m_t = nc.dram_tensor("m_t", [KH, 128, BPC], FP32, kind="ExternalInput")
    out = nc.dram_tensor("out", [BPC, H], FP32, kind="ExternalOutput")

    wt_d = nc.inline_tensor(Wt, name="wt")            # [8, 128, 1536] fp32
    f1_d = nc.inline_tensor(F1t, name="f1t")          # [16, 128, 120]
    f2_d = nc.inline_tensor(F2t, name="f2t")          # [120, 1]
    id_d = nc.inline_tensor(np.eye(128, dtype=np.float32), name="ident")
    if use_fc1_bias:
        f1b_d = nc.inline_tensor(fc1_b.reshape(SH, 1).astype(np.float32),
                                 name="f1b")
    if use_gru_bias:
        ones_d = nc.inline_tensor(np.ones((1, 128), np.float32), name="ones1")
        bx_d = nc.inline_tensor(b_ih.reshape(1, G3).astype(np.float32),
                                name="bx")
        bh_d = nc.inline_tensor(b_hh.reshape(1, G3).astype(np.float32),
                                name="bh")

    g_scr = nc.dram_tensor("g_scr", [S, BPC], FP32)   # internal scratch

    with tile.TileContext(nc) as tc:
        with (
            tc.tile_pool(name="const", bufs=1) as cpool,
            tc.tile_pool(name="state", bufs=3) as hpool,
            tc.tile_pool(name="work", bufs=2) as wpool,
        ):
            # ---- constants to SBUF ----
            wt = cpool.tile([128, 8, G3], FP32R, tag="wt")
            nc.sync.dma_start(
                wt[:], wt_d.ap().rearrange("k h g -> h k g").bitcast(FP32R))
            f1t = cpool.tile([128, 16, SH], FP32R, tag="f1t")
            nc.sync.dma_start(
                f1t[:], f1_d.ap().rearrange("k h o -> h k o").bitcast(FP32R))
            f2t = cpool.tile([SH, 1], FP32R, tag="f2t")
            nc.sync.dma_start(f2t[:], f2_d.ap().bitcast(FP32R))
            idt = cpool.tile([128, 128], FP32, tag="idt")
            nc.sync.dma_start(idt[:], id_d.ap())
            qm = cpool.tile([128, KH, BPC], FP32, tag="qm")
            nc.sync.dma_start(qm[:], q_t.ap().rearrange("k h b -> h k b"))
            mm = cpool.tile([128, KH, BPC], FP32, tag="mm")
            nc.sync.dma_start(mm[:], m_t.ap().rearrange("k h b -> h k b"))
            if use_fc1_bias:
                f1b = cpool.tile([SH, 1], FP32, tag="f1b")
                nc.sync.dma_start(f1b[:], f1b_d.ap())
            if use_gru_bias:
                onest = cpool.tile([1, 128], FP32R, tag="ones1")
                nc.sync.dma_start(onest[:], ones_d.ap().bitcast(FP32R))
                bxt = cpool.tile([1, G3], FP32R, tag="bx")
                nc.sync.dma_start(bxt[:], bx_d.ap().bitcast(FP32R))
                bht = cpool.tile([1, G3], FP32R, tag="bh")
                nc.sync.dma_start(bht[:], bh_d.ap().bitcast(FP32R))

            # ---- phase 1: scoring network -> G in g_scr[s, b] ----
            with (
                tc.tile_pool(name="p1sb", bufs=2) as p1sb,
                tc.tile_pool(name="feat", bufs=6) as fpool,
                tc.tile_pool(name="p1ps", bufs=2, space="PSUM") as p1ps,
            ):
                for gi in range(NGRP):
                    s0 = gi * SGRP
                    cg = p1sb.tile([128, KH, SGRP, BPC], FP32R, tag="cg")
                    for k in range(KH):
                        nc.sync.dma_start(
                            cg[:, k],
                            c_t.ap()[s0:s0 + SGRP, k].rearrange(
                                "s h b -> h s b"))
                    pps = p1ps.tile([SH, SGRP * BPC], FP32, tag="pps")
                    for k in range(16):
                        kc = k % KH
                        which = k // KH  # 0: C*Q, 1: C*M, 2: |C-Q|, 3: |C-M|
                        fk = fpool.tile([128, SGRP, BPC], FP32R, tag="feat")
                        cgk = cg[:, kc]
                        if which == 0 or which == 1:
                            other = qm if which == 0 else mm
                            ob = other[:, kc].unsqueeze(1).broadcast_to(
                                [128, SGRP, BPC])
                            nc.vector.tensor_tensor(
                                fk[:], cgk.bitcast(FP32), ob, OP.mult)
                        else:
                            other = qm if which == 2 else mm
                            ob = other[:, kc].unsqueeze(1).broadcast_to(
                                [128, SGRP, BPC])
                            dtmp = p1sb.tile([128, SGRP, BPC], FP32,
                                             tag="dtmp")
                            nc.vector.tensor_tensor(
                                dtmp[:], cgk.bitcast(FP32), ob, OP.subtract)
                            nc.scalar.activation(fk[:], dtmp[:], AF.Abs)
                        nc.tensor.matmul(
                            pps[:], f1t[:, k], fk[:],
                            start=(k == 0), stop=(k == 15))
                    h1 = p1sb.tile([SH, SGRP * BPC], FP32R, tag="h1")
                    if use_fc1_bias:
                        nc.scalar.activation(h1[:], pps[:], AF.Tanh,
                                             bias=f1b[:, 0:1])
                    else:
                        nc.scalar.activation(h1[:], pps[:], AF.Tanh)
                    pg = p1ps.tile([1, SGRP * BPC], FP32, tag="pg")
                    nc.tensor.matmul(pg[:], f2t[:], h1[:],
                                     start=True, stop=True)
                    gt = p1sb.tile([1, SGRP * BPC], FP32, tag="gt")
                    nc.scalar.activation(gt[:], pg[:], AF.Sigmoid,
                                         bias=fc2_bias)
                    nc.sync.dma_start(g_scr.ap()[s0:s0 + SGRP], gt[:])

            # gather G to [b, s] + negated copy
            gsb = cpool.tile([BPC, S], FP32, tag="gsb")
            nc.sync.dma_start(gsb[:], g_scr.ap().rearrange("s b -> b s"))
            ngsb = cpool.tile([BPC, S], FP32, tag="ngsb")
            nc.vector.tensor_scalar_mul(ngsb[:], gsb[:], -1.0)

            # ---- phase 2: gated GRU scan ----
            h_prev = hpool.tile([BPC, H], FP32, tag="h")
            nc.vector.memset(h_prev[:], 0.0)

            with (
                tc.tile_pool(name="cs", bufs=3) as cspool,
                tc.tile_pool(name="ht", bufs=2) as htpool,
                tc.tile_pool(name="ew", bufs=2) as ew,
                tc.tile_pool(name="ew1", bufs=2) as ew1,
                tc.tile_pool(name="ps_r", bufs=2, space="PSUM") as ps_r,
                tc.tile_pool(name="ps_z", bufs=2, space="PSUM") as ps_z,
                tc.tile_pool(name="ps_n", bufs=1, space="PSUM") as ps_n,
                tc.tile_pool(name="ps_h", bufs=1, space="PSUM") as ps_h,
                tc.tile_pool(name="ps_t", bufs=1, space="PSUM") as ps_t,
            ):
                for s in range(S):
                    cst = cspool.tile([128, KH, BPC], FP32R, tag="cs")
                    nc.sync.dma_start(
                        cst[:], c_t.ap()[s].rearrange("k h b -> h k b"))

                    if s > 0:
                        pt = ps_t.tile([128, H], FP32, tag="pt")
                        for k in range(KH):
                            nc.tensor.transpose(
                                pt[:, k * 128:(k + 1) * 128],
                                h_prev[:, k * 128:(k + 1) * 128], idt[:])
                        ht = htpool.tile([128, KH, BPC], FP32R, tag="ht")
                        nc.scalar.activation(ht[:], pt[:].rearrange(
                            "h (k b) -> h k b", k=KH), AF.Copy)

                    # matmuls: pr = xr+hr, pz = xz+hz, pxn = xn, phn = hn
                    pr = ps_r.tile([BPC, H], FP32, tag="pr")
                    pz = ps_z.tile([BPC, H], FP32, tag="pz")
                    pxn = ps_n.tile([BPC, H], FP32, tag="pxn")
                    if s > 0:
                        phn = ps_h.tile([BPC, H], FP32, tag="phn")
                    else:
                        phn = None
                    for ci, psum in ((0, pr), (1, pz), (2, pxn)):
                        c0 = ci * H
                        nmm = KH if (ci == 2 or s == 0) else 2 * KH
                        i = 0
                        for k in range(KH):
                            nc.tensor.matmul(
                                psum[:], cst[:, k], wt[:, k, c0:c0 + H],
                                start=(i == 0),
                                stop=(i == nmm - 1 and not use_gru_bias))
                            i += 1
                        if s > 0 and ci != 2:
                            for k in range(KH):
                                nc.tensor.matmul(
                                    psum[:], ht[:, k], wt[:, KH + k, c0:c0 + H],
                                    start=False,
                                    stop=(i == nmm - 1 and not use_gru_bias))
                                i += 1
                        if use_gru_bias:
                            nc.tensor.matmul(
                                psum[:], onest[:], bxt[:, c0:c0 + H],
                                start=False, stop=(ci == 2 and s == 0))
                            if ci != 2:
                                nc.tensor.matmul(
                                    psum[:], onest[:], bht[:, c0:c0 + H],
                                    start=False, stop=True)
                    if s > 0:
                        for k in range(KH):
                            nc.tensor.matmul(
                                phn[:], ht[:, k], wt[:, KH + k, 2 * H:G3],
                                start=(k == 0),
                                stop=(k == KH - 1 and not use_gru_bias))
                        if use_gru_bias:
                            nc.tensor.matmul(
                                phn[:], onest[:], bht[:, 2 * H:G3],
                                start=False, stop=True)

                    # elementwise update
                    r_sb = ew.tile([BPC, H], FP32, tag="r")
                    nc.scalar.activation(r_sb[:], pr[:], AF.Sigmoid)
                    z_sb = ew.tile([BPC, H], FP32, tag="z")
                    nc.scalar.activation(z_sb[:], pz[:], AF.Sigmoid)
                    a_sb = ew.tile([BPC, H], FP32, tag="a")
                    nc.vector.tensor_scalar(
                        a_sb[:], z_sb[:], ngsb[:, s:s + 1], gsb[:, s:s + 1],
                        OP.mult, OP.add)
                    n_sb = ew.tile([BPC, H], FP32, tag="n")
                    if s > 0:
                        tn = ew1.tile([BPC, H], FP32, tag="tn")
                        nc.vector.tensor_tensor(tn[:], r_sb[:], phn[:],
                                                OP.mult)
                        tn2 = ew1.tile([BPC, H], FP32, tag="tn2")
                        nc.vector.tensor_tensor(tn2[:], tn[:], pxn[:], OP.add)
                        nc.scalar.activation(n_sb[:], tn2[:], AF.Tanh)
                    else:
                        nc.scalar.activation(n_sb[:], pxn[:], AF.Tanh)
                    t_sb = ew1.tile([BPC, H], FP32, tag="t")
                    nc.vector.tensor_sub(t_sb[:], n_sb[:], h_prev[:])
                    u_sb = ew1.tile([BPC, H], FP32, tag="u")
                    nc.vector.tensor_mul(u_sb[:], a_sb[:], t_sb[:])
                    h_new = hpool.tile([BPC, H], FP32, tag="h")
                    nc.vector.tensor_add(h_new[:], h_prev[:], u_sb[:])
                    h_prev = h_new

                nc.sync.dma_start(out.ap(), h_prev[:])

    nc.compile()
    return nc


def _prep(C, Q, prev_M, fc1_w, fc2_w, W_ih, W_hh):
    """Host-side sharding + layout transforms."""
    Wt = np.concatenate([
        np.ascontiguousarray(W_ih.T).reshape(KH, 128, G3),
        np.ascontiguousarray(W_hh.T).reshape(KH, 128, G3),
    ], axis=0).astype(np.float32)
    F1t = np.ascontiguousarray(fc1_w.T).reshape(16, 128, SH).astype(np.float32)
    F2t = np.ascontiguousarray(fc2_w.T).astype(np.float32)  # [120, 1]

    in_maps = []
    for c in range(NCORES):
        lo, hi = c * BPC, (c + 1) * BPC
        c_sh = C[lo:hi]                    # [128, 64, 512]
        c_tr = np.ascontiguousarray(c_sh.transpose(1, 2, 0)).reshape(
            S, KH, 128, BPC).astype(np.float32)
        q_tr = np.ascontiguousarray(Q[lo:hi, 0].T).reshape(
            KH, 128, BPC).astype(np.float32)
        m_tr = np.ascontiguousarray(prev_M[lo:hi, 0].T).reshape(
            KH, 128, BPC).astype(np.float32)
        in_maps.append({"c_t": c_tr, "q_t": q_tr, "m_t": m_tr})
    return Wt, F1t, F2t, in_maps


def kernel(C, Q, prev_M, fc1_w, fc1_b, fc2_w, fc2_b, W_ih, W_hh, b_ih, b_hh):
    from concourse.bass_utils import run_bass_kernel_spmd

    C = np.asarray(C, dtype=np.float32)
    Q = np.asarray(Q, dtype=np.float32)
    prev_M = np.asarray(prev_M, dtype=np.float32)
    Wt, F1t, F2t, in_maps = _prep(C, Q, prev_M,
                                  np.asarray(fc1_w, np.float32),
                                  np.asarray(fc2_w, np.float32),
                                  np.asarray(W_ih, np.float32),
                                  np.asarray(W_hh, np.float32))

    key = (Wt.tobytes(), F1t.tobytes(), F2t.tobytes(),
           np.asarray(fc1_b).tobytes(), np.asarray(fc2_b).tobytes(),
           np.asarray(b_ih).tobytes(), np.asarray(b_hh).tobytes())
    kh = hash(key)
    if kh not in _CACHE:
        _CACHE[kh] = _build(Wt, F1t, F2t,
                            np.asarray(fc1_b, np.float32),
                            np.asarray(fc2_b, np.float32),
                            np.asarray(b_ih, np.float32),
                            np.asarray(b_hh, np.float32))
    nc = _CACHE[kh]

    res = run_bass_kernel_spmd(nc, in_maps, list(range(NCORES)))
    h = np.concatenate([res.results[c]["out"] for c in range(NCORES)], axis=0)
    return h[:, None, :].astype(np.float32)


# revision 9
# speedup vs baseline: 1.3539x; 1.3539x over previous
"""EpisodicMemory Trainium2 kernel.

Data-parallel over batch across 8 NeuronCores (128 batch rows per core).
Per-core program (SPMD, weights baked into the NEFF as inline constants):

  Scoring: G[b,s] = sigmoid(fc2(tanh(fc1(feat)))),
    feat = [C*Q, C*prev_M, |C-Q|, |C-prev_M|], built in bf16 in a
    feature-on-partitions layout; fc1 is a bf16 matmul with N=512.
    Scoring work is interleaved into the scan (one group of 4 sentences
    spread over 4 scan steps, two groups of lookahead) so it fills
    engine idle time created by the scan's serial dependence.

  Scan (gated GRU over s): per step the PE computes
    [C_s; h] @ [W_ih; W_hh].T in fp32r (r/z fused; n split because r
    multiplies only the hidden half), state h kept [batch-part, H-free],
    h transposed on the PE each step to feed the next matmul. Update:
    h_new = h + g*(1-z)*(n - h), with per-partition gate scalars.
    The r-path and the chain tail are split into column halves so the
    serial chain pipelines across engines; subs/abs/gating run on the
    otherwise-idle GPSIMD engine.
"""
import numpy as np

H = 512
SH = 120
B = 1024
S = 64
NCORES = 8
BPC = B // NCORES  # 128
KH = H // 128      # 4
G3 = 3 * H
SGRP = 4
NGRP = S // SGRP   # 16
HHALF = H // 2     # 256

_CACHE = {}


def _build(Wt, F1t, F2t, fc1_b, fc2_b, b_ih, b_hh):
    import concourse.bass as bass
    import concourse.tile as tile
    from concourse import bacc, mybir

    FP32 = mybir.dt.float32
    FP32R = mybir.dt.float32r
    BF16 = mybir.dt.bfloat16
    U16 = mybir.dt.uint16
    AF = mybir.ActivationFunctionType
    OP = mybir.AluOpType

    use_gru_bias = bool(np.any(b_ih != 0) or np.any(b_hh != 0))
    use_fc1_bias = bool(np.any(fc1_b != 0))
    fc2_bias = float(np.asarray(fc2_b).reshape(-1)[0])

    nc = bacc.Bacc("TRN2", target_bir_lowering=False, debug=False,
                   num_devices=NCORES)

    c_t = nc.dram_tensor("c_t", [S, KH, 128, BPC], FP32R, kind="ExternalInput")
    c_b = nc.dram_tensor("c_b", [S, KH, 128, BPC], BF16, kind="ExternalInput")
    q_t = nc.dram_tensor("q_t", [KH, 128, BPC], BF16, kind="ExternalInput")
    m_t = nc.dram_tensor("m_t", [KH, 128, BPC], BF16, kind="ExternalInput")
    out = nc.dram_tensor("out", [BPC, H], FP32, kind="ExternalOutput")

    wt_d = nc.inline_tensor(Wt, name="wt")              # [8,128,1536] fp32
    f1_d = nc.inline_tensor(F1t.astype(np.float32), name="f1t")
    f2_d = nc.inline_tensor(F2t.astype(np.float32), name="f2t")
    id_d = nc.inline_tensor(np.eye(128, dtype=np.float32), name="ident")
    if use_fc1_bias:
        f1b_d = nc.inline_tensor(fc1_b.reshape(SH, 1).astype(np.float32),
                                 name="f1b")
    if use_gru_bias:
        ones_d = nc.inline_tensor(np.ones((1, 128), np.float32), name="ones1")
        bx_d = nc.inline_tensor(b_ih.reshape(1, G3).astype(np.float32),
                                name="bx")
        bh_d = nc.inline_tensor(b_hh.reshape(1, G3).astype(np.float32),
                                name="bh")

    with tile.TileContext(nc) as tc:
        with (
            tc.tile_pool(name="const", bufs=1) as cpool,
            tc.tile_pool(name="state", bufs=3) as hpool,
            tc.tile_pool(name="p1sb", bufs=2) as p1sb,
            tc.tile_pool(name="feat", bufs=10) as fpool,
            tc.tile_pool(name="gtile", bufs=4) as gpool,
            tc.tile_pool(name="gdram", bufs=4, space="DRAM") as gdram,
            tc.tile_pool(name="cs", bufs=3) as cspool,
            tc.tile_pool(name="cbs", bufs=3) as cbpool,
            tc.tile_pool(name="ht", bufs=2) as htpool,
            tc.tile_pool(name="ew", bufs=2) as ew,
            tc.tile_pool(name="ew1", bufs=2) as ew1,
            tc.tile_pool(name="ps_r", bufs=2, space="PSUM") as ps_r,
            tc.tile_pool(name="ps_z", bufs=2, space="PSUM") as ps_z,
            tc.tile_pool(name="ps_n", bufs=1, space="PSUM") as ps_n,
            tc.tile_pool(name="ps_h", bufs=1, space="PSUM") as ps_h,
            tc.tile_pool(name="ps_t", bufs=1, space="PSUM") as ps_t,
            tc.tile_pool(name="ps_f", bufs=1, space="PSUM") as ps_f,
        ):
            # ---- constants ----
            wt = cpool.tile([128, 8, G3], FP32R, tag="wt")
            nc.sync.dma_start(
                wt[:], wt_d.ap().rearrange("k h g -> h k g").bitcast(FP32R))
            f1t = cpool.tile([128, 16, SH], BF16, tag="f1t")
            f1f = cpool.tile([128, 16, SH], FP32, tag="f1f")
            nc.sync.dma_start(f1f[:], f1_d.ap().rearrange("k h o -> h k o"))
            nc.vector.tensor_copy(f1t[:], f1f[:])
            f2t = cpool.tile([SH, 1], BF16, tag="f2t")
            f2f = cpool.tile([SH, 1], FP32, tag="f2f")
            nc.sync.dma_start(f2f[:], f2_d.ap())
            nc.vector.tensor_copy(f2t[:], f2f[:])
            idt = cpool.tile([128, 128], FP32, tag="idt")
            nc.sync.dma_start(idt[:], id_d.ap())
            qm = cpool.tile([128, KH, BPC], BF16, tag="qm")
            nc.sync.dma_start(qm[:], q_t.ap().rearrange("k h b -> h k b"))
            mm = cpool.tile([128, KH, BPC], BF16, tag="mm")
            nc.sync.dma_start(mm[:], m_t.ap().rearrange("k h b -> h k b"))
            # materialized broadcast of Q/M over the 4 sentences of a group
            qrep = cpool.tile([128, KH, SGRP, BPC], BF16, tag="qrep")
            mrep = cpool.tile([128, KH, SGRP, BPC], BF16, tag="mrep")
            for k in range(KH):
                nc.vector.tensor_copy(
                    qrep[:, k],
                    qm[:, k].unsqueeze(1).broadcast_to([128, SGRP, BPC]))
                nc.vector.tensor_copy(
                    mrep[:, k],
                    mm[:, k].unsqueeze(1).broadcast_to([128, SGRP, BPC]))
            if use_fc1_bias:
                f1b = cpool.tile([SH, 1], FP32, tag="f1b")
                nc.sync.dma_start(f1b[:], f1b_d.ap())
            if use_gru_bias:
                onest = cpool.tile([1, 128], FP32R, tag="ones1")
                nc.sync.dma_start(onest[:], ones_d.ap().bitcast(FP32R))
                bxt = cpool.tile([1, G3], FP32R, tag="bx")
                nc.sync.dma_start(bxt[:], bx_d.ap().bitcast(FP32R))
                bht = cpool.tile([1, G3], FP32R, tag="bh")
                nc.sync.dma_start(bht[:], bh_d.ap().bitcast(FP32R))

            # ---- scoring group machinery (interleaved into the scan) ----
            grp_state = {}   # gi -> dict with cgb, pps, g4, ng4

            def load_group(gi):
                cgb = cbpool.tile([128, KH, SGRP, BPC], BF16, tag="cgb")
                s0 = gi * SGRP
                for k in range(KH):
                    nc.sync.dma_start(
                        cgb[:, k],
                        c_b.ap()[s0:s0 + SGRP, k].rearrange("s h b -> h s b"))
                grp_state[gi] = {"cgb": cgb}

            def emit_group_quarter(gi, q):
                """Emit feat k-tiles 4q..4q+3 + their fc1 matmuls."""
                st = grp_state[gi]
                cgb = st["cgb"]
                if q == 0:
                    st["pps"] = ps_f.tile([SH, SGRP * BPC], FP32, tag="pps",
                                          name="pps")
                pps = st["pps"]
                # one k-tile of each feat component per quarter so the
                # per-step DVE/GPSIMD load stays even
                for i, k in enumerate((q, 4 + q, 8 + q, 12 + q)):
                    kc = k % KH
                    which = k // KH
                    fk = fpool.tile([128, SGRP, BPC], BF16, tag="feat")
                    cgk = cgb[:, kc]
                    rep = qrep if which in (0, 2) else mrep
                    if which == 0:
                        nc.vector.tensor_tensor(
                            fk[:], cgk, rep[:, kc], OP.mult)
                    elif which == 1:
                        nc.gpsimd.tensor_tensor(
                            fk[:], cgk, rep[:, kc], OP.mult)
                    else:
                        dt_ = p1sb.tile([128, SGRP, BPC], BF16, tag="dtmp")
                        nc.gpsimd.tensor_tensor(
                            dt_[:], cgk, rep[:, kc], OP.subtract)
                        nc.vector.tensor_scalar(
                            fk[:].bitcast(U16), dt_[:].bitcast(U16),
                            0x7FFF, None, OP.bitwise_and)
                    nc.tensor.matmul(pps[:], f1t[:, k], fk[:],
                                     start=(i == 0 and q == 0),
                                     stop=(i == 3 and q == 3))

            def finish_group(gi):
                st = grp_state[gi]
                pps = st["pps"]
                h1 = p1sb.tile([SH, SGRP * BPC], BF16, tag="h1")
                if use_fc1_bias:
                    nc.scalar.activation(h1[:], pps[:], AF.Tanh,
                                         bias=f1b[:, 0:1])
                else:
                    nc.scalar.activation(h1[:], pps[:], AF.Tanh)
                nc.tensor.matmul(pps[0:1, :], f2t[:], h1[:],
                                 start=True, stop=True)
                gt = gpool.tile([1, SGRP * BPC], FP32, tag="gt")
                nc.scalar.activation(gt[:], pps[0:1, :], AF.Sigmoid,
                                     bias=fc2_bias)
                gd = gdram.tile([SGRP, BPC], FP32, tag="gd")
                nc.sync.dma_start(gd[:], gt[:])
                g4 = gpool.tile([BPC, SGRP], FP32, tag="g4")
                nc.sync.dma_start(g4[:], gd[:].rearrange("s b -> b s"))
                st["g4"] = g4
                del st["cgb"], st["pps"]

            # prologue: groups 0 and 1 fully
            for gi in (0, 1):
                load_group(gi)
                for q in range(4):
                    emit_group_quarter(gi, q)
                finish_group(gi)

            # initial state: two column halves
            h_half = []
            for hf in range(2):
                t0 = hpool.tile([BPC, HHALF], FP32, tag=f"h{hf}", name=f"h{hf}")
                nc.vector.memset(t0[:], 0.0)
                h_half.append(t0)

            for s in range(S):
                # --- C_s load (fp32r, for matmul lhsT) ---
                cst = cspool.tile([128, KH, BPC], FP32R, tag="cs")
                nc.sync.dma_start(
                    cst[:], c_t.ap()[s].rearrange("k h b -> h k b"))

                # --- x-part matmuls (no h dependence) ---
                prh = [ps_r.tile([BPC, HHALF], FP32, tag="prh", name="prh") for _ in range(2)]
                pz = ps_z.tile([BPC, H], FP32, tag="pz")
                pxn = ps_n.tile([BPC, H], FP32, tag="pxn")
                n_r = KH if s == 0 else 2 * KH
                for hf in range(2):
                    c0 = hf * HHALF
                    for k in range(KH):
                        nc.tensor.matmul(
                            prh[hf][:], cst[:, k], wt[:, k, c0:c0 + HHALF],
                            start=(k == 0),
                            stop=(k == n_r - 1 and not use_gru_bias))
                for k in range(KH):
                    nc.tensor.matmul(
                        pz[:], cst[:, k], wt[:, k, H:2 * H],
                        start=(k == 0),
                        stop=(k == n_r - 1 and not use_gru_bias))
                for k in range(KH):
                    nc.tensor.matmul(
                        pxn[:], cst[:, k], wt[:, k, 2 * H:G3],
                        start=(k == 0),
                        stop=(k == KH - 1 and not use_gru_bias))
                if use_gru_bias:
                    for hf in range(2):
                        c0 = hf * HHALF
                        nc.tensor.matmul(prh[hf][:], onest[:],
                                         bxt[:, c0:c0 + HHALF],
                                         start=False, stop=(s == 0))
                        if s == 0:
                            nc.tensor.matmul(prh[hf][:], onest[:],
                                             bht[:, c0:c0 + HHALF],
                                             start=False, stop=False)
                    nc.tensor.matmul(pz[:], onest[:], bxt[:, H:2 * H],
                                     start=False, stop=(s == 0))
                    if s == 0:
                        nc.tensor.matmul(pz[:], onest[:], bht[:, H:2 * H],
                                         start=False, stop=False)
                    nc.tensor.matmul(pxn[:], onest[:], bxt[:, 2 * H:G3],
                                     start=False, stop=True)

                # --- interleaved scoring work ---
                gi = s // SGRP + 2
                q = s % SGRP
                if gi <= NGRP - 1:
                    if q == 0:
                        load_group(gi)
                    emit_group_quarter(gi, q)
                    if q == 3:
                        finish_group(gi)

                # --- h-dependent matmuls ---
                if s > 0:
                    pt = ps_t.tile([128, H], FP32, tag="pt")
                    for k in range(KH):
                        nc.tensor.transpose(
                            pt[:, k * 128:(k + 1) * 128],
                            h_half[k // 2][:, (k % 2) * 128:(k % 2) * 128 + 128],
                            idt[:])
                    ht = htpool.tile([128, KH, BPC], FP32R, tag="ht")
                    for hf in range(2):
                        nc.scalar.activation(
                            ht[:, 2 * hf:2 * hf + 2],
                            pt[:, hf * 256:(hf + 1) * 256].rearrange(
                                "h (k b) -> h k b", k=2), AF.Copy)
                    phn = ps_h.tile([BPC, H], FP32, tag="phn")
                    for hf in range(2):
                        c0 = hf * HHALF
                        for k in range(KH):
                            nc.tensor.matmul(
                                prh[hf][:], ht[:, k], wt[:, KH + k, c0:c0 + HHALF],
                                start=False,
                                stop=(k == KH - 1 and not use_gru_bias))
                    for k in range(KH):
                        nc.tensor.matmul(
                            pz[:], ht[:, k], wt[:, KH + k, H:2 * H],
                            start=False,
                            stop=(k == KH - 1 and not use_gru_bias))
                    for k in range(KH):
                        nc.tensor.matmul(
                            phn[:], ht[:, k], wt[:, KH + k, 2 * H:G3],
                            start=(k == 0),
                            stop=(k == KH - 1 and not use_gru_bias))
                    if use_gru_bias:
                        for hf in range(2):
                            nc.tensor.matmul(
                                prh[hf][:], onest[:],
                                bht[:, hf * HHALF:(hf + 1) * HHALF],
                                start=False, stop=True)
                        nc.tensor.matmul(pz[:], onest[:], bht[:, H:2 * H],
                                         start=False, stop=True)
                        nc.tensor.matmul(phn[:], onest[:], bht[:, 2 * H:G3],
                                         start=False, stop=True)

                # --- elementwise update (r-path and tail in column halves) ---
                gst = grp_state[s // SGRP]
                g4 = gst["g4"]
                j = s % SGRP

                r_h = []
                for hf in range(2):
                    rh = ew.tile([BPC, HHALF], FP32, tag=f"r{hf}", name=f"r{hf}")
                    nc.scalar.activation(rh[:], prh[hf][:], AF.Sigmoid)
                    r_h.append(rh)
                # w = 1 - z = sigmoid(-pz)
                w_sb = ew.tile([BPC, H], FP32, tag="w")
                nc.scalar.activation(w_sb[:], pz[:], AF.Sigmoid, scale=-1.0)

                new_h = []
                for hf in range(2):
                    c0 = hf * HHALF
                    n_hf = ew1.tile([BPC, HHALF], FP32, tag=f"n{hf}", name=f"n{hf}")
                    if s > 0:
                        tn = ew1.tile([BPC, HHALF], FP32, tag=f"tn{hf}", name=f"tn{hf}")
                        nc.vector.tensor_tensor(
                            tn[:], r_h[hf][:], phn[:, c0:c0 + HHALF], OP.mult)
                        tn2 = ew1.tile([BPC, HHALF], FP32, tag=f"tn2{hf}", name=f"tn2{hf}")
                        nc.vector.tensor_tensor(
                            tn2[:], tn[:], pxn[:, c0:c0 + HHALF], OP.add)
                        nc.scalar.activation(n_hf[:], tn2[:], AF.Tanh)
                    else:
                        nc.scalar.activation(n_hf[:], pxn[:, c0:c0 + HHALF],
                                             AF.Tanh)
                    # h_new = h + g*w*(n - h)
                    t_hf = ew1.tile([BPC, HHALF], FP32, tag=f"t{hf}", name=f"t{hf}")
                    nc.vector.tensor_sub(t_hf[:], n_hf[:], h_half[hf][:])
                    u_hf = ew1.tile([BPC, HHALF], FP32, tag=f"u{hf}", name=f"u{hf}")
                    nc.vector.tensor_mul(u_hf[:], w_sb[:, c0:c0 + HHALF],
                                         t_hf[:])
                    nh = hpool.tile([BPC, HHALF], FP32, tag=f"h{hf}", name=f"h{hf}")
                    nc.vector.scalar_tensor_tensor(
                        nh[:], u_hf[:], g4[:, j:j + 1], h_half[hf][:],
                        OP.mult, OP.add)
                    new_h.append(nh)
                h_half = new_h

            nc.sync.dma_start(out.ap()[:, 0:HHALF], h_half[0][:])
            nc.sync.dma_start(out.ap()[:, HHALF:H], h_half[1][:])

    nc.compile()
    return nc


def _prep(C, Q, prev_M, fc1_w, fc2_w, W_ih, W_hh):
    """Host-side sharding + layout transforms."""
    Wt = np.concatenate([
        np.ascontiguousarray(W_ih.T).reshape(KH, 128, G3),
        np.ascontiguousarray(W_hh.T).reshape(KH, 128, G3),
    ], axis=0).astype(np.float32)
    F1t = np.ascontiguousarray(fc1_w.T).reshape(16, 128, SH).astype(np.float32)
    F2t = np.ascontiguousarray(fc2_w.T).astype(np.float32)  # [120, 1]

    import ml_dtypes
    in_maps = []
    for c in range(NCORES):
        lo, hi = c * BPC, (c + 1) * BPC
        c_tr = np.ascontiguousarray(C[lo:hi].transpose(1, 2, 0)).reshape(
            S, KH, 128, BPC).astype(np.float32)
        q_tr = np.ascontiguousarray(Q[lo:hi, 0].T).reshape(
            KH, 128, BPC).astype(ml_dtypes.bfloat16)
        m_tr = np.ascontiguousarray(prev_M[lo:hi, 0].T).reshape(
            KH, 128, BPC).astype(ml_dtypes.bfloat16)
        in_maps.append({
            "c_t": c_tr,
            "c_b": c_tr.astype(ml_dtypes.bfloat16),
            "q_t": q_tr,
            "m_t": m_tr,
        })
    return Wt, F1t, F2t, in_maps


def kernel(C, Q, prev_M, fc1_w, fc1_b, fc2_w, fc2_b, W_ih, W_hh, b_ih, b_hh):
    from concourse.bass_utils import run_bass_kernel_spmd

    C = np.asarray(C, dtype=np.float32)
    Q = np.asarray(Q, dtype=np.float32)
    prev_M = np.asarray(prev_M, dtype=np.float32)
    Wt, F1t, F2t, in_maps = _prep(C, Q, prev_M,
                                  np.asarray(fc1_w, np.float32),
                                  np.asarray(fc2_w, np.float32),
                                  np.asarray(W_ih, np.float32),
                                  np.asarray(W_hh, np.float32))

    key = (Wt.tobytes(), F1t.tobytes(), F2t.tobytes(),
           np.asarray(fc1_b).tobytes(), np.asarray(fc2_b).tobytes(),
           np.asarray(b_ih).tobytes(), np.asarray(b_hh).tobytes())
    kh = hash(key)
    if kh not in _CACHE:
        _CACHE[kh] = _build(Wt, F1t, F2t,
                            np.asarray(fc1_b, np.float32),
                            np.asarray(fc2_b, np.float32),
                            np.asarray(b_ih, np.float32),
                            np.asarray(b_hh, np.float32))
    nc = _CACHE[kh]

    res = run_bass_kernel_spmd(nc, in_maps, list(range(NCORES)))
    h = np.concatenate([res.results[c]["out"] for c in range(NCORES)], axis=0)
    return h[:, None, :].astype(np.float32)
